# revision 9
# baseline (speedup 1.0000x reference)
"""BiDAF attention kernel for 8 Trainium2 NeuronCores (data-parallel over batch).

Contract: kernel(**inputs) takes the FULL unsharded inputs (as produced by the
reference setup_inputs) and returns the FULL [16, 1024, 2048] fp32 output.

Math (per batch b):
    s[i,j]  = c[i].c_w + q[j].q_w + sum_h c[i,h]*cqw[h]*q[j,h] + bias
    s1      = softmax_j(masked(s, q_mask));  s2 = softmax_i(masked(s, c_mask))
    a       = s1 @ q ; bb = s1 @ s2^T @ c
    out     = concat(c, a, c*a, c*bb)

v2 design (DMA-byte-minimized — DMA is the roofline at ~360 GB/s/core):
  - Host folds cq_weight/c_weight into qw'[j,h] = q*cqw + c_w so one matmul
    chain gives sT[j,i] = sim_cq[i,j] + sim_c[i]; sim_q + bias + q_mask fold
    into the Exp activation's per-partition bias (f32).
  - Host uploads fp16 copies of BOTH orientations of c (cT for the similarity
    matmul, row-major c for traw/cr1) — no PE transposes of c on device.
  - One exp(sT) serves both softmaxes; normalizations are per-partition scales
    folded into the downstream matmul outputs. e is kept in bf16 (values reach
    e^~12, which overflows fp16; f32/f32r can't dtype-mix per the walrus
    verifier), always as the *stationary* matmul operand; moving operands
    (ct, q, cr, t) are fp16 — bf16/f16 matmuls stream at 1 cycle/row.
  - Outputs a|c*a and c*b are written as fp16 and upcast to f32 on the host
    (tolerance is 2e-2; fp16 rounding is ~5e-4).
  - The c block of the output is assembled on the host (pure copy of an input).
"""

import os
import sys
from contextlib import ExitStack

import numpy as np

for _p in ("/opt/trn_rl_repo", "/root/.axon_site/_ro/trn_rl_repo"):
    if os.path.isdir(_p) and _p not in sys.path:
        sys.path.append(_p)

B, CL, QL, H = 16, 1024, 128, 512
N_CORES = 8
BPC = B // N_CORES  # batches per core
KT = H // 128  # 4 k-tiles over the hidden dim
IT = CL // 128  # 8 i-tiles over the context dim
NEG = np.float32(-1e30)
NEG16 = np.float16(-30000.0)  # exp(s + NEG16) == 0 exactly in f32

_build_cache = {}


def _build(mask_trivial: bool):
    key = (mask_trivial,)
    if key in _build_cache:
        return _build_cache[key]

    import concourse.bass as bass
    import concourse.tile as tile
    from concourse import bacc, mybir

    F32 = mybir.dt.float32
    BF16 = mybir.dt.bfloat16
    F16 = mybir.dt.float16
    AF = mybir.ActivationFunctionType
    PSUM = bass.MemorySpace.PSUM

    nc = bacc.Bacc("TRN2", target_bir_lowering=False, debug=False)

    # fp16 inputs, host-prepped:
    #  ct: c transposed -> partition = h%128 of k-tile, free = (k, i)
    #  cr: c row-major  -> partition = i%128 of i-tile, free = (it, h)
    #  qpk: [0:512] qw'T (partition h%128, free (k,j)); [512:1024] q (partition j);
    #       [1024:1026] qbias f32 bitcast as 2 f16 halves (per-partition j)
    ct_d = nc.dram_tensor("ct", [BPC, 128, KT * CL], F16, kind="ExternalInput")
    cr_d = nc.dram_tensor("cr", [BPC, 128, IT * H], F16, kind="ExternalInput")
    qpk_d = nc.dram_tensor("qpk", [BPC, 128, 1026], F16, kind="ExternalInput")
    if not mask_trivial:
        cmask_d = nc.dram_tensor("cmaskb", [BPC, 1, CL], F16, kind="ExternalInput")
        onesr_d = nc.dram_tensor("onesr", [1, QL], F16, kind="ExternalInput")
    aca_d = nc.dram_tensor("out_aca", [BPC, CL, 2 * H], F16, kind="ExternalOutput")
    cb_d = nc.dram_tensor("out_cb", [BPC, CL, H], F16, kind="ExternalOutput")

    with tile.TileContext(nc) as tc, ExitStack() as ctx:
        const = ctx.enter_context(tc.tile_pool(name="const", bufs=1))
        sbp = ctx.enter_context(tc.tile_pool(name="sbp", bufs=2))
        outp = ctx.enter_context(tc.tile_pool(name="outp", bufs=6))
        ps_acc = ctx.enter_context(tc.tile_pool(name="ps_acc", bufs=2, space=PSUM))
        ps_tr = ctx.enter_context(tc.tile_pool(name="ps_tr", bufs=1, space=PSUM))
        ps_ab = ctx.enter_context(tc.tile_pool(name="ps_ab", bufs=4, space=PSUM))

        if not mask_trivial:
            cmask_all = const.tile([1, BPC * CL], F16, tag="cmask")
            nc.sync.dma_start(cmask_all[:], cmask_d.ap().rearrange("b one i -> one (b i)"))
            onesr = const.tile([1, QL], F16, tag="onesr")
            nc.sync.dma_start(onesr[:], onesr_d.ap())

        # ---- PE clock warmup + ACT exp-table preload in the preamble window.
        warmf = const.tile([128, 1], F32, tag="warmf")
        nc.vector.memset(warmf[:], 0.0)
        nc.scalar.activation(warmf[:, 0:1], warmf[:, 0:1], AF.Exp)
        warmL = const.tile([128, 1], F16, tag="warmL")
        warmC = const.tile([128, 512], F16, tag="warmC")
        nc.vector.memset(warmL[:], 0.0)
        nc.vector.memset(warmC[:], 0.0)
        pw = ps_tr.tile([128, 512], F32, tag="tr")
        for _ in range(8):
            nc.tensor.matmul(pw[:1, :], warmL[:], warmC[:], start=True, stop=True)

        # ---- phase A: emit ALL loads (both batches) on the SP HWDGE queue.
        LD = []
        for bi in range(BPC):
            qpk = sbp.tile([128, 1026], F16, tag="qpk")
            nc.sync.dma_start(qpk[:], qpk_d.ap()[bi])
            ct = sbp.tile([128, KT, CL], F16, tag="ct")
            nc.sync.dma_start(ct[:], ct_d.ap()[bi].rearrange("p (k i) -> p k i", k=KT))
            cr = sbp.tile([128, IT, H], F16, tag="cr")
            nc.sync.dma_start(cr[:], cr_d.ap()[bi].rearrange("p (t h) -> p t h", t=IT))
            LD.append((qpk, ct, cr))

        # ---- phase B1: per-batch similarity front-end + a|c*a stores ----
        ST = []
        for bi in range(BPC):
            qpk, ct, cr = LD[bi]
            qwT = qpk[:, 0:512].rearrange("p (k j) -> p k j", k=KT)
            q_sb = qpk[:, 512:1024]
            qbias = qpk[:, 1024:1026].bitcast(F32)

            ehalf = []
            r1h = []
            eN = []
            rs2 = sbp.tile([QL, 2], F32, tag="rs2")
            for nh in range(2):
                spt = ps_acc.tile([QL, 512], F32, tag="acc")
                for k in range(KT):
                    nc.tensor.matmul(
                        spt[:],
                        qwT[:, k, :],
                        ct[:, k, nh * 512 : (nh + 1) * 512],
                        start=(k == 0),
                        stop=(k == KT - 1 and mask_trivial),
                    )
                if not mask_trivial:
                    nc.tensor.matmul(
                        spt[:],
                        onesr[:],
                        cmask_all[:, bi * CL + nh * 512 : bi * CL + (nh + 1) * 512],
                        start=False,
                        stop=True,
                    )

                eh = sbp.tile([QL, 512], BF16, tag=f"e{nh}")
                nc.scalar.activation(
                    eh[:],
                    spt[:],
                    AF.Exp,
                    bias=qbias[:],
                    scale=1.0,
                    accum_out=rs2[:, nh : nh + 1],
                )
                ehalf.append(eh)

                # transpose e via the DMA XBAR (keeps the PE free):
                # eNh[p, b, j] = eh[j, b*128+p], i.e. i on partitions
                eNh = sbp.tile([128, 4, 128], BF16, tag=f"eN{nh}")
                nc.sync.dma_start_transpose(eNh[:], eh[:])
                eN.append(eNh)

                # r1 denominators: free-axis reduce of transposed e
                sums = sbp.tile([128, 4], F32, tag=f"s1{nh}")
                for j in range(4):
                    nc.vector.tensor_reduce(
                        sums[:, j : j + 1],
                        eNh[:, j, :],
                        mybir.AxisListType.X,
                        mybir.AluOpType.add,
                    )
                r1n = sbp.tile([128, 4], F32, tag=f"r1{nh}")
                nc.vector.reciprocal(r1n[:], sums[:])

                # a | c*a for this half's i-tiles
                for j in range(4):
                    it = 4 * nh + j
                    esl = eh[:, j * 128 : (j + 1) * 128]
                    pa = ps_ab.tile([128, H], F32, tag="ab")
                    nc.tensor.matmul(pa[:], esl, q_sb, start=True, stop=True)
                    aca_sb = outp.tile([128, 2 * H], F16, tag="aca")
                    if it % 2 == 0:
                        nc.scalar.mul(aca_sb[:, 0:H], pa[:], r1n[:, j : j + 1])
                    else:
                        nc.vector.tensor_scalar_mul(aca_sb[:, 0:H], pa[:], r1n[:, j : j + 1])
                    nc.gpsimd.tensor_mul(aca_sb[:, H : 2 * H], aca_sb[:, 0:H], cr[:, it, :])
                    rows = aca_d.ap()[bi, it * 128 : (it + 1) * 128]
                    nc.scalar.dma_start(rows[:], aca_sb[:])
                r1h.append(r1n)

            ST.append((cr, ehalf, r1h, eN, rs2))

        # ---- phase B2: per-batch b path: r2, eN, traw, t, c*b stores ----
        for bi in range(BPC):
            cr, ehalf, r1h, eN, rs2 = ST[bi]
            rsum = sbp.tile([QL, 1], F32, tag="rsum")
            nc.vector.tensor_reduce(rsum[:], rs2[:], mybir.AxisListType.X, mybir.AluOpType.add)
            r2 = sbp.tile([QL, 1], F32, tag="r2")
            nc.vector.reciprocal(r2[:], rsum[:])

            ptraw = ps_acc.tile([QL, H], F32, tag="acc")
            for it in range(IT):
                nc.tensor.matmul(
                    ptraw[:],
                    eN[it // 4][:, it % 4, :],
                    cr[:, it, :],
                    start=(it == 0),
                    stop=(it == IT - 1),
                )
            t_sb = sbp.tile([QL, H], F16, tag="t")
            nc.scalar.mul(t_sb[:], ptraw[:], r2[:])

            for it in range(IT):
                esl = ehalf[it // 4][:, (it % 4) * 128 : (it % 4 + 1) * 128]
                pb = ps_ab.tile([128, H], F32, tag="ab")
                nc.tensor.matmul(pb[:], esl, t_sb[:], start=True, stop=True)
                rb = r1h[it // 4][:, it % 4 : it % 4 + 1]
                bb = sbp.tile([128, H], F16, tag="bb", bufs=3)
                if it % 2 == 0:
                    nc.vector.tensor_scalar_mul(bb[:], pb[:], rb)
                else:
                    nc.scalar.mul(bb[:], pb[:], rb)
                cb_sb = outp.tile([128, H], F16, tag="cb")
                nc.gpsimd.tensor_mul(cb_sb[:], bb[:], cr[:, it, :])
                rows = cb_d.ap()[bi, it * 128 : (it + 1) * 128]
                nc.sync.dma_start(rows[:], cb_sb[:])

    nc.compile()
    _build_cache[key] = nc
    return nc


def _install_profshim():
    """Optional NTFF profiling support (BIDAF_PROFILE=1); self-contained."""
    import contextlib
    import ctypes
    import types

    if "antenv.axon_hooks" in sys.modules:
        return
    so_path = "/opt/axon/libaxon_pjrt.so"
    try:
        lib = ctypes.CDLL(so_path)
    except OSError:
        return
    if not hasattr(lib, "axon_start_nrt_profile"):
        return
    lib.axon_start_nrt_profile.argtypes = [ctypes.POINTER(ctypes.c_int64), ctypes.c_size_t]
    lib.axon_start_nrt_profile.restype = ctypes.c_int64
    lib.axon_stop_nrt_profile.argtypes = [ctypes.c_char_p]
    lib.axon_stop_nrt_profile.restype = ctypes.c_int64

    @contextlib.contextmanager
    def _hook(output_dir, device_ids):
        import jax

        jax.devices()
        if device_ids:
            ids = (ctypes.c_int64 * len(device_ids))(*device_ids)
            rc = lib.axon_start_nrt_profile(ids, len(device_ids))
        else:
            rc = lib.axon_start_nrt_profile(None, 0)
        if rc != 0:
            raise RuntimeError(f"axon_start_nrt_profile rc={rc}")
        try:
            yield
        finally:
            n = lib.axon_stop_nrt_profile(str(output_dir).encode())
            print(f"profile: {n} file(s) written to {output_dir}")

    mod = types.ModuleType("antenv.axon_hooks")
    mod.get_axon_ntff_profile_hook = lambda: _hook
    mod.set_axon_ntff_profile_hook = lambda h: None
    sys.modules["antenv.axon_hooks"] = mod
    import antenv

    antenv.axon_hooks = mod

    from concourse import bass_utils

    bass_utils.upload_artifacts = lambda tmpdir: f"local:{tmpdir}"


def kernel(c, q, c_mask, q_mask, c_weight, q_weight, cq_weight, bias):
    from concourse.bass_utils import run_bass_kernel_spmd

    c = np.asarray(c, dtype=np.float32)
    q = np.asarray(q, dtype=np.float32)
    c_mask = np.asarray(c_mask)
    q_mask = np.asarray(q_mask)
    c_weight = np.asarray(c_weight, dtype=np.float32)
    q_weight = np.asarray(q_weight, dtype=np.float32)
    cq_weight = np.asarray(cq_weight, dtype=np.float32)
    bias = np.asarray(bias, dtype=np.float32)

    # host-side folding (all tiny, O(B*(CL+QL)*H) at most)
    qw = q * cq_weight.reshape(1, 1, H) + c_weight.reshape(1, 1, H)  # [B, QL, H]
    sim_q = (q @ q_weight)[:, :, 0]  # [B, QL]
    amask_q = (1.0 - q_mask.astype(np.float32)) * NEG
    qbias = (sim_q + bias[0] + amask_q).astype(np.float32)  # [B, QL]
    amask_c = ((1.0 - c_mask.astype(np.float32)) * NEG16).reshape(B, 1, CL)
    mask_trivial = bool((amask_c == 0).all())

    qpk = np.empty((B, 128, 1026), dtype=np.float16)
    qpk[:, :, 0:512] = (
        qw.reshape(B, QL, KT, 128).transpose(0, 3, 2, 1).reshape(B, 128, KT * QL)
    )
    qpk[:, :, 512:1024] = q
    qpk[:, :, 1024:1026] = np.ascontiguousarray(qbias).reshape(B, QL, 1).view(np.float16)

    ct = (
        c.transpose(0, 2, 1).reshape(B, KT, 128, CL).transpose(0, 2, 1, 3).reshape(B, 128, KT * CL)
    ).astype(np.float16)
    cr = (
        c.reshape(B, IT, 128, H).transpose(0, 2, 1, 3).reshape(B, 128, IT * H)
    ).astype(np.float16)

    profile = os.environ.get("BIDAF_PROFILE", "") == "1"
    if profile:
        _install_profshim()

    nc = _build(mask_trivial)

    onesr = np.ones((1, QL), dtype=np.float16)
    in_maps = []
    for core in range(N_CORES):
        s = slice(BPC * core, BPC * (core + 1))
        m = {
            "ct": np.ascontiguousarray(ct[s]),
            "cr": np.ascontiguousarray(cr[s]),
            "qpk": np.ascontiguousarray(qpk[s]),
        }
        if not mask_trivial:
            m["cmaskb"] = np.ascontiguousarray(amask_c[s].astype(np.float16))
            m["onesr"] = onesr
        in_maps.append(m)

    kw = {}
    if profile:
        kw = dict(trace=True, tmpdir=os.environ.get("BIDAF_PROFILE_DIR") or None)
    res = run_bass_kernel_spmd(nc, in_maps, list(range(N_CORES)), **kw)
    if profile and res.exec_time_ns is not None:
        print(f"[kernel] HW exec time: {res.exec_time_ns} ns")
        kernel.last_exec_time_ns = res.exec_time_ns
        kernel.last_trace = res.instructions_and_trace[1] if res.instructions_and_trace else None

    out = np.empty((B, CL, 4 * H), dtype=np.float32)
    out[:, :, 0:H] = c
    for i in range(N_CORES):
        out[BPC * i : BPC * (i + 1), :, H : 3 * H] = res.results[i]["out_aca"].astype(np.float32)
        out[BPC * i : BPC * (i + 1), :, 3 * H :] = res.results[i]["out_cb"].astype(np.float32)
    return out


kernel.last_exec_time_ns = None
kernel.last_trace = None
